# revision 24
# baseline (speedup 1.0000x reference)
"""Multi-head attention (B=4, T=2048, D=1024, H=16 heads, causal) on 8
Trainium2 NeuronCores.

Sharding: core i handles batch b = i//2 and head-group g = i%2 (8 heads,
512 features). Each core computes its head-group's attention output and a
partial output projection; the host sums the two partials per batch.

All heavy matmuls run as float32r (fp32 storage, ~12-bit-mantissa multiply)
which streams at full PE rate for free-dim >= 256.

Attention is computed entirely in "transposed score" layout to avoid any
on-device transposes:
  ST[k, q] = K_k . Q_q   (keys on partitions, queries on free dim)
  P^T = exp(ST/8 - M0) * causal_mask   (ACT exp, per-partition bias slot)
  [OT; s] = V'^T @ P^T   where V' has a ones column, so the softmax
            denominator s comes out of the same matmul (row 64).
  OT_norm = OT * (1/s)   (DVE reciprocal + PE rank-1 broadcast of 1/s)
  y_partial = OT_norm^T @ Wo^T-slice  (natural layout, DMA'd out directly)
"""

import numpy as np

import concourse.bass as bass
import concourse.mybir as mybir
import concourse.tile as tile
from concourse import bass_utils

import orjson

D_MODEL = 1024
N_HEADS = 16
D_K = 64
B, T = 4, 2048
FG = 512            # features per head-group (8 heads x 64)
N_CORES = 8
M0 = 12.0           # global exp shift (cancels exactly in softmax)

F32R = mybir.dt.float32r
F32 = mybir.dt.float32
AF = mybir.ActivationFunctionType


# ---------------------------------------------------------------------------
# BIR legalization for the stock walrus in this container: it encodes at most
# ONE sync wait per instruction, so spill extra waits onto single-wait
# EventSemaphore nops placed immediately before the instruction on the same
# engine.  Also statically verify no wait precedes (in program order) the
# instructions that produce its semaphore value, which would deadlock.
# ---------------------------------------------------------------------------

def split_multiwaits(bir_bytes: bytes) -> bytes:
    bir = orjson.loads(bir_bytes)
    n = [0]
    for fn in bir.get("functions", []):
        for blk in fn.get("blocks", []):
            out = []
            for inst in blk.get("instructions", []):
                si = inst.get("sync_info") or {}
                waits = si.get("on_wait") or []
                if len(waits) > 1:
                    for w in waits[:-1]:
                        n[0] += 1
                        out.append({
                            "debug": inst.get("debug", 0),
                            "engine": inst["engine"],
                            "ins": [], "name": f"WSPLIT-{n[0]}",
                            "opcode": "EventSemaphore", "outs": [],
                            "sync_info": {"on_update": [], "on_wait": [w]},
                        })
                    si["on_wait"] = waits[-1:]
                    inst["sync_info"] = si
                out.append(inst)
            blk["instructions"] = out
    _check_no_forward_waits(bir)
    return orjson.dumps(bir)


def _check_no_forward_waits(bir) -> None:
    issued = {}
    for fn in bir.get("functions", []):
        for blk in fn.get("blocks", []):
            for inst in blk.get("instructions", []):
                si = inst.get("sync_info") or {}
                for w in (si.get("on_wait") or []):
                    if (w.get("sync_type") == "semaphore"
                            and w.get("wait_mode") == "sem-ge-imm"
                            and "barrier" not in (w.get("ant_name") or "")):
                        if issued.get(w["id"], 0) < w["wait_value"]:
                            raise RuntimeError(
                                f"forward wait: {inst['name']} waits "
                                f"{w.get('ant_name')}>={w['wait_value']}")
                for u in (si.get("on_update") or []):
                    if (u.get("sync_type") == "semaphore"
                            and u.get("update_mode") in ("sem-inc", "sem-add-imm")):
                        issued[u["id"]] = issued.get(u["id"], 0) + u.get("update_value", 1)


# ---------------------------------------------------------------------------
# Device program (identical on all 8 cores; data differs per core)
# ---------------------------------------------------------------------------

def build_nc(st_bufs=7, psa_bufs=5, exp_batch=1, batch_act=False, pool_mask=False, psb_bufs=2, psc_bufs=1, msc_bufs=2, act_width=512, y_from_psum=False, y_defer=False, psp_bufs=8, j_order=None, causal=True) -> bass.Bass:
    nc = bass.Bass("TRN2", target_bir_lowering=False, debug=False)

    xt = nc.dram_tensor("xt", (D_MODEL, T), F32R, kind="ExternalInput")
    wq = nc.dram_tensor("wq", (D_MODEL, FG), F32R, kind="ExternalInput")
    wk = nc.dram_tensor("wk", (D_MODEL, FG), F32R, kind="ExternalInput")
    wv = nc.dram_tensor("wv", (D_MODEL, FG), F32R, kind="ExternalInput")
    wo = nc.dram_tensor("wo", (FG, D_MODEL), F32R, kind="ExternalInput")
    cvec = nc.dram_tensor("cvec", (T,), F32, kind="ExternalInput")
    onesd = nc.dram_tensor("onesd", (1, 64), F32R, kind="ExternalInput")
    onescol = nc.dram_tensor("onescol", (128, 16), F32R, kind="ExternalInput")
    maskd = nc.dram_tensor("maskd", (128, 512), F32R, kind="ExternalInput")
    y = nc.dram_tensor("y", (T, D_MODEL), F32, kind="ExternalOutput")

    NDT = D_MODEL // 128   # 8 contraction tiles
    NFT = FG // 128        # 4 feature tiles (2 heads each)
    NTT = T // 128         # 16 token tiles
    NTS = T // 512         # 4 token strips

    with tile.TileContext(nc) as tc, nc.allow_low_precision(reason="f32r storage"):
        with (
            tc.tile_pool(name="qtp", bufs=1) as qtp,
            tc.tile_pool(name="ktp", bufs=1) as ktp,
            tc.tile_pool(name="vpp", bufs=1) as vpp,
            tc.tile_pool(name="stp", bufs=st_bufs) as stp,
            tc.tile_pool(name="cst", bufs=1) as cst,
            tc.tile_pool(name="msc", bufs=msc_bufs) as msc,
        ):
            # persistent tiles
            qt = [qtp.tile([128, T], F32R, name=f"qt{i}") for i in range(NFT)]
            kt = [ktp.tile([128, T], F32R, name=f"kt{i}") for i in range(NFT)]
            vp = [vpp.tile([128, 520], F32R, name=f"vp{i}") for i in range(NTT)]
            cb = cst.tile([128, 16], F32, name="cb")
            ones = cst.tile([1, 64], F32R, name="ones")
            mask = cst.tile([128, 512], F32R, name="mask")
            nc.sync.dma_start(cb[:], cvec.rearrange("(a b) -> b a", b=128))
            nc.sync.dma_start(ones[:], onesd[:])
            nc.sync.dma_start(mask[:], maskd[:])

            # ---- phase 1: projections ------------------------------------
            with (
                tc.tile_pool(name="xtp", bufs=1) as xtp,
                tc.tile_pool(name="wp", bufs=1) as wp,
                tc.tile_pool(name="psP", bufs=psp_bufs, space="PSUM") as psA,
            ):
                xts = [xtp.tile([128, T], F32R, name=f"xts{i}") for i in range(NDT)]
                wq_t = [wp.tile([128, FG], F32R, name=f"w{i}", tag=f"w{i}")
                        for i in range(NDT)]
                # interleave weight tiles with strip-0 x tiles so the
                # first psum accumulation can start after ~0.5MB of DMA
                for i in range(NDT):
                    nc.sync.dma_start(wq_t[i][:], wq[i * 128:(i + 1) * 128, :])
                    nc.sync.dma_start(
                        xts[i][:, 0:512], xt[i * 128:(i + 1) * 128, 0:512])
                for ts in range(1, NTS):
                    for i in range(NDT):
                        nc.sync.dma_start(
                            xts[i][:, ts * 512:(ts + 1) * 512],
                            xt[i * 128:(i + 1) * 128, ts * 512:(ts + 1) * 512])

                def emit_qk(wtl, dst):
                    for ts in range(NTS):
                        for ft in range(NFT):
                            ps = psA.tile([128, 512], F32, name="pj", tag="A")
                            for d in range(NDT):
                                nc.tensor.matmul(
                                    ps[:], wtl[d][:, ft * 128:(ft + 1) * 128],
                                    xts[d][:, ts * 512:(ts + 1) * 512],
                                    start=(d == 0), stop=(d == NDT - 1))
                            nc.vector.tensor_copy(
                                dst[ft][:, ts * 512:(ts + 1) * 512], ps[:])

                # Q^T first, then V (attention j=0 needs V'[0:4]), K^T last:
                # attention starts as soon as K^T strip 0 lands.
                emit_qk(wq_t, qt)

                wvl = [wp.tile([128, FG], F32R, name=f"wv{i}", tag=f"w{i}")
                       for i in range(NDT)]
                for i in range(NDT):
                    nc.sync.dma_start(wvl[i][:], wv[i * 128:(i + 1) * 128, :])
                for tt in range(NTT):
                    ps = psA.tile([128, 512], F32, name="pv", tag="A")
                    for d in range(NDT):
                        nc.tensor.matmul(
                            ps[:], xts[d][:, tt * 128:(tt + 1) * 128], wvl[d][:],
                            start=(d == 0), stop=(d == NDT - 1))
                    vv = vp[tt].rearrange("p (h c) -> p h c", h=8)
                    pv = ps.rearrange("p (h c) -> p h c", h=8)
                    nc.vector.tensor_copy(vv[:, :, 0:64], pv[:])
                    nc.sync.dma_start(
                        vv[:, :, 64:65],
                        onescol.rearrange("p (h c) -> p h c", h=16)[:, 0:8, :])

                wkl = [wp.tile([128, FG], F32R, name=f"wk{i}", tag=f"w{i}")
                       for i in range(NDT)]
                for i in range(NDT):
                    nc.sync.dma_start(wkl[i][:], wk[i * 128:(i + 1) * 128, :])
                emit_qk(wkl, kt)

            # ---- phase 2: attention + output projection ------------------
            with (
                tc.tile_pool(name="otp", bufs=1) as otp,
                tc.tile_pool(name="wop", bufs=1) as wop,
                tc.tile_pool(name="psA", bufs=psa_bufs, space="PSUM") as psA,
                tc.tile_pool(name="psB", bufs=psb_bufs, space="PSUM") as psB,
                tc.tile_pool(name="psC", bufs=psc_bufs, space="PSUM") as psC,
            ):
                ot = [otp.tile([128, T], F32R, name=f"ot{i}") for i in range(NFT)]
                wot = [wop.tile([128, D_MODEL], F32R, name=f"wot{i}")
                       for i in range(NFT)]
                for i in range(NFT):
                    nc.sync.dma_start(wot[i][:], wo[i * 128:(i + 1) * 128, :])

                for j in (j_order or range(NTS)):
                    for h in range(8):
                        ft, base = h // 2, (h % 2) * 64
                        if causal:
                            nkt = 4 * j + 4    # causal key tiles for strip j
                            korder = list(range(4 * j, nkt)) + list(range(0, 4 * j))
                        else:
                            nkt = NTT
                            korder = list(range(NTT))
                        ops = psB.tile([128, 512], F32, name="ops", tag="B")

                        def _lo(kt_):
                            if not causal:
                                return 0
                            m_ = kt_ - 4 * j
                            return 128 * m_ if m_ > 0 else 0

                        nb = act_width // 512
                        ki = 0
                        while ki < nkt:
                            gts = korder[ki:ki + nb]
                            if any(_lo(k_) for k_ in gts):
                                gts = gts[:1]
                            gn = len(gts)
                            lo0 = _lo(gts[0])
                            sps = psA.tile([128, act_width], F32, name="sps", tag="A")
                            se = stp.tile([128, act_width], F32R, name="se", tag="se")
                            for u, ktile in enumerate(gts):
                                lo = _lo(ktile)
                                nc.tensor.matmul(
                                    sps[:, u * 512 + lo:(u + 1) * 512],
                                    kt[ft][base:base + 64, ktile * 128:(ktile + 1) * 128],
                                    qt[ft][base:base + 64, j * 512 + lo:(j + 1) * 512],
                                    start=True, stop=True)
                            if gn == nb and nb > 1:
                                nc.scalar.activation(
                                    se[:, 0:gn * 512], sps[:, 0:gn * 512], AF.Exp,
                                    bias=cb[:, gts[0]:gts[0] + 1], scale=0.125)
                            else:
                                for u, ktile in enumerate(gts):
                                    lo = _lo(ktile)
                                    nc.scalar.activation(
                                        se[:, u * 512 + lo:(u + 1) * 512],
                                        sps[:, u * 512 + lo:(u + 1) * 512], AF.Exp,
                                        bias=cb[:, ktile:ktile + 1], scale=0.125)
                            for u, ktile in enumerate(gts):
                                m = (ktile - 4 * j) if causal else -1
                                if m >= 0:
                                    lo = _lo(ktile)
                                    nc.vector.tensor_mul(
                                        se[:, u * 512 + lo:(u + 1) * 512],
                                        se[:, u * 512 + lo:(u + 1) * 512],
                                        mask[:, 0: 512 - 128 * m])
                            for u, ktile in enumerate(gts):
                                lo = _lo(ktile)
                                nc.tensor.matmul(
                                    ops[0:65, lo:512],
                                    vp[ktile][:, h * 65:(h + 1) * 65],
                                    se[:, u * 512 + lo:(u + 1) * 512],
                                    start=(ki + u == 0), stop=(ki + u == nkt - 1))
                            ki += gn
                        # normalize rows 0:64 by row 64 and store to OT
                        r = msc.tile([1, 512], F32R, name="r", tag="r")
                        nc.vector.reciprocal(r[:], ops[64:65, :])
                        rps = psC.tile([64, 512], F32, name="rps", tag="C")
                        nc.tensor.matmul(rps[:], ones[:], r[:],
                                         start=True, stop=True)
                        rb = msc.tile([64, 512], F32, name="rb", tag="rb")
                        nc.vector.tensor_copy(rb[:], rps[:])
                        nc.vector.tensor_mul(
                            ot[ft][base:base + 64, j * 512:(j + 1) * 512],
                            ops[0:64, :], rb[:])

                    # y = OT^T @ woT; optionally deferred one strip to
                    # dodge the diagonal-heavy DVE window
                    yjs = ([j - 1] if j > 0 else []) if y_defer else [j]
                    if y_defer and j == NTS - 1:
                        yjs = [j - 1, j]
                    for yj in yjs:
                      for tt in range(4 * yj, 4 * yj + 4):
                        for nn in range(2):
                            yps = psC.tile([128, 512], F32, name="yps", tag="C")
                            for ft in range(NFT):
                                nc.tensor.matmul(
                                    yps[:], ot[ft][:, tt * 128:(tt + 1) * 128],
                                    wot[ft][:, nn * 512:(nn + 1) * 512],
                                    start=(ft == 0), stop=(ft == NFT - 1))
                            ysb = msc.tile([128, 512], F32, name="ysb", tag="y")
                            nc.vector.tensor_copy(ysb[:], yps[:])
                            nc.sync.dma_start(
                                y[tt * 128:(tt + 1) * 128, nn * 512:(nn + 1) * 512],
                                ysb[:])

    _orig = nc.to_json_bytes
    nc.to_json_bytes = lambda: split_multiwaits(_orig())
    return nc


_NC = {}


def _get_nc(causal=True) -> bass.Bass:
    if causal not in _NC:
        _NC[causal] = build_nc(causal=causal)
    return _NC[causal]


# ---------------------------------------------------------------------------
# Host-side sharding + gather
# ---------------------------------------------------------------------------

def _kernel_numpy(q, mask, Wq, bq, Wk, bk, Wv, bv, Wo, bo):
    """Exact host fallback for unexpected shapes or arbitrary masks."""
    b, t, d = q.shape
    h = N_HEADS if d == D_MODEL else max(1, d // D_K)
    dk = d // h
    qh = (q @ Wq.T + bq).reshape(b, t, h, dk).transpose(0, 2, 1, 3)
    kh = (q @ Wk.T + bk).reshape(b, t, h, dk).transpose(0, 2, 1, 3)
    vh = (q @ Wv.T + bv).reshape(b, t, h, dk).transpose(0, 2, 1, 3)
    s = np.einsum("bhqd,bhkd->bhqk", qh, kh) / np.sqrt(dk).astype(np.float32)
    s = np.where(mask, -np.inf, s)
    s = s - s.max(axis=-1, keepdims=True)
    p = np.exp(s)
    p /= p.sum(axis=-1, keepdims=True)
    o = np.einsum("bhqk,bhkd->bhqd", p, vh)
    o = o.transpose(0, 2, 1, 3).reshape(b, t, d)
    return (o @ Wo.T + bo).astype(np.float32)


_CAUSAL_REF = None


def _mask_kind(mask):
    """Classify the (B,1,T,T) bool mask: "causal" / "none" / "other"."""
    global _CAUSAL_REF
    mask = np.asarray(mask)
    if mask.shape != (B, 1, T, T):
        return "other"
    if not mask.any():
        return "none"
    if _CAUSAL_REF is None:
        _CAUSAL_REF = np.triu(np.ones((T, T), dtype=bool), k=1)
    for i in range(mask.shape[0]):
        if not np.array_equal(mask[i, 0], _CAUSAL_REF):
            return "other"
    return "causal"


def kernel(q, mask, Wq, bq, Wk, bk, Wv, bv, Wo, bo):
    q = np.asarray(q, np.float32)
    Wq = np.asarray(Wq, np.float32); bq = np.asarray(bq, np.float32)
    Wk = np.asarray(Wk, np.float32); bk = np.asarray(bk, np.float32)
    Wv = np.asarray(Wv, np.float32); bv = np.asarray(bv, np.float32)
    Wo = np.asarray(Wo, np.float32); bo = np.asarray(bo, np.float32)

    kind = _mask_kind(mask)
    if q.shape != (B, T, D_MODEL) or Wq.shape != (D_MODEL, D_MODEL) or kind == "other":
        return _kernel_numpy(np.asarray(q, np.float32), np.asarray(mask, bool),
                             Wq, bq, Wk, bk, Wv, bv, Wo, bo)

    onesd = np.ones((1, 64), np.float32)
    onescol = np.ones((128, 16), np.float32)
    # causal staircase: M[k, u] = 1 iff k <= u (used as [0 : 512-128m])
    kk = np.arange(128)[:, None]
    vv_ = np.arange(512)[None, :]
    maskst = (kk <= vv_).astype(np.float32)

    in_maps = []
    for b in range(B):
        x = np.ascontiguousarray(q[b])                    # (T, D)
        xT = np.ascontiguousarray(x.T)                    # (D, T)
        # per-key exp bias: bq . K_k term (softmax-relevant) minus shift M0
        cvec = ((x @ (Wk.T @ bq) + float(bq @ bk)) * 0.125 - M0).astype(np.float32)
        for g in range(2):
            sl = slice(g * FG, (g + 1) * FG)
            in_maps.append({
                "xt": xT,
                "wq": np.ascontiguousarray(Wq[sl].T),     # (D, FG)
                "wk": np.ascontiguousarray(Wk[sl].T),
                "wv": np.ascontiguousarray(Wv[sl].T),
                "wo": np.ascontiguousarray(Wo[:, sl].T),  # (FG, D)
                "cvec": cvec,
                "onesd": onesd,
                "onescol": onescol,
                "maskd": maskst,
            })

    nc = _get_nc(causal=(kind == "causal"))
    res = bass_utils.run_bass_kernel_spmd(nc, in_maps, core_ids=list(range(N_CORES)))

    out = np.empty((B, T, D_MODEL), np.float32)
    # exact bias correction terms (zero when biases are zero):
    # V-bias contributes bv @ Wo.T (softmax rows sum to 1); plus bo.
    corr = (bv @ Wo.T + bo).astype(np.float32)
    for b in range(B):
        out[b] = res.results[2 * b]["y"] + res.results[2 * b + 1]["y"] + corr
    return out


# revision 26
# speedup vs baseline: 1.0009x; 1.0009x over previous
"""Multi-head attention (B=4, T=2048, D=1024, H=16 heads, causal) on 8
Trainium2 NeuronCores.

Sharding: core i handles batch b = i//2 and head-group g = i%2 (8 heads,
512 features). Each core computes its head-group's attention output and a
partial output projection; the host sums the two partials per batch.

All heavy matmuls run as float32r (fp32 storage, ~12-bit-mantissa multiply)
which streams at full PE rate for free-dim >= 256.

Attention is computed entirely in "transposed score" layout to avoid any
on-device transposes:
  ST[k, q] = K_k . Q_q   (keys on partitions, queries on free dim)
  P^T = exp(ST/8 - M0) * causal_mask   (ACT exp, per-partition bias slot)
  [OT; s] = V'^T @ P^T   where V' has a ones column, so the softmax
            denominator s comes out of the same matmul (row 64).
  OT_norm = OT * (1/s)   (DVE reciprocal + PE rank-1 broadcast of 1/s)
  y_partial = OT_norm^T @ Wo^T-slice  (natural layout, DMA'd out directly)
"""

import numpy as np

import concourse.bass as bass
import concourse.mybir as mybir
import concourse.tile as tile
from concourse import bass_utils

import orjson

D_MODEL = 1024
N_HEADS = 16
D_K = 64
B, T = 4, 2048
FG = 512            # features per head-group (8 heads x 64)
N_CORES = 8
M0 = 12.0           # global exp shift (cancels exactly in softmax)
KORDER_MODE = "diag_last"

F32R = mybir.dt.float32r
F32 = mybir.dt.float32
AF = mybir.ActivationFunctionType


# ---------------------------------------------------------------------------
# BIR legalization for the stock walrus in this container: it encodes at most
# ONE sync wait per instruction, so spill extra waits onto single-wait
# EventSemaphore nops placed immediately before the instruction on the same
# engine.  Also statically verify no wait precedes (in program order) the
# instructions that produce its semaphore value, which would deadlock.
# ---------------------------------------------------------------------------

def split_multiwaits(bir_bytes: bytes) -> bytes:
    bir = orjson.loads(bir_bytes)
    n = [0]
    for fn in bir.get("functions", []):
        for blk in fn.get("blocks", []):
            out = []
            for inst in blk.get("instructions", []):
                si = inst.get("sync_info") or {}
                waits = si.get("on_wait") or []
                if len(waits) > 1:
                    for w in waits[:-1]:
                        n[0] += 1
                        out.append({
                            "debug": inst.get("debug", 0),
                            "engine": inst["engine"],
                            "ins": [], "name": f"WSPLIT-{n[0]}",
                            "opcode": "EventSemaphore", "outs": [],
                            "sync_info": {"on_update": [], "on_wait": [w]},
                        })
                    si["on_wait"] = waits[-1:]
                    inst["sync_info"] = si
                out.append(inst)
            blk["instructions"] = out
    _check_no_forward_waits(bir)
    return orjson.dumps(bir)


def _check_no_forward_waits(bir) -> None:
    issued = {}
    for fn in bir.get("functions", []):
        for blk in fn.get("blocks", []):
            for inst in blk.get("instructions", []):
                si = inst.get("sync_info") or {}
                for w in (si.get("on_wait") or []):
                    if (w.get("sync_type") == "semaphore"
                            and w.get("wait_mode") == "sem-ge-imm"
                            and "barrier" not in (w.get("ant_name") or "")):
                        if issued.get(w["id"], 0) < w["wait_value"]:
                            raise RuntimeError(
                                f"forward wait: {inst['name']} waits "
                                f"{w.get('ant_name')}>={w['wait_value']}")
                for u in (si.get("on_update") or []):
                    if (u.get("sync_type") == "semaphore"
                            and u.get("update_mode") in ("sem-inc", "sem-add-imm")):
                        issued[u["id"]] = issued.get(u["id"], 0) + u.get("update_value", 1)


# ---------------------------------------------------------------------------
# Device program (identical on all 8 cores; data differs per core)
# ---------------------------------------------------------------------------

def build_nc(st_bufs=7, psa_bufs=5, exp_batch=1, batch_act=False, pool_mask=False, psb_bufs=2, psc_bufs=1, msc_bufs=2, act_width=512, y_from_psum=False, y_defer=False, psp_bufs=8, j_order=None, causal=True) -> bass.Bass:
    nc = bass.Bass("TRN2", target_bir_lowering=False, debug=False)

    xt = nc.dram_tensor("xt", (D_MODEL, T), F32R, kind="ExternalInput")
    wq = nc.dram_tensor("wq", (D_MODEL, FG), F32R, kind="ExternalInput")
    wk = nc.dram_tensor("wk", (D_MODEL, FG), F32R, kind="ExternalInput")
    wv = nc.dram_tensor("wv", (D_MODEL, FG), F32R, kind="ExternalInput")
    wo = nc.dram_tensor("wo", (FG, D_MODEL), F32R, kind="ExternalInput")
    cvec = nc.dram_tensor("cvec", (T,), F32, kind="ExternalInput")
    onesd = nc.dram_tensor("onesd", (1, 64), F32R, kind="ExternalInput")
    onescol = nc.dram_tensor("onescol", (128, 16), F32R, kind="ExternalInput")
    maskd = nc.dram_tensor("maskd", (128, 512), F32R, kind="ExternalInput")
    y = nc.dram_tensor("y", (T, D_MODEL), F32, kind="ExternalOutput")

    NDT = D_MODEL // 128   # 8 contraction tiles
    NFT = FG // 128        # 4 feature tiles (2 heads each)
    NTT = T // 128         # 16 token tiles
    NTS = T // 512         # 4 token strips

    with tile.TileContext(nc) as tc, nc.allow_low_precision(reason="f32r storage"):
        with (
            tc.tile_pool(name="qtp", bufs=1) as qtp,
            tc.tile_pool(name="ktp", bufs=1) as ktp,
            tc.tile_pool(name="vpp", bufs=1) as vpp,
            tc.tile_pool(name="stp", bufs=st_bufs) as stp,
            tc.tile_pool(name="cst", bufs=1) as cst,
            tc.tile_pool(name="msc", bufs=msc_bufs) as msc,
        ):
            # persistent tiles
            qt = [qtp.tile([128, T], F32R, name=f"qt{i}") for i in range(NFT)]
            kt = [ktp.tile([128, T], F32R, name=f"kt{i}") for i in range(NFT)]
            vp = [vpp.tile([128, 520], F32R, name=f"vp{i}") for i in range(NTT)]
            cb = cst.tile([128, 16], F32, name="cb")
            ones = cst.tile([1, 64], F32R, name="ones")
            mask = cst.tile([128, 512], F32R, name="mask")
            nc.sync.dma_start(cb[:], cvec.rearrange("(a b) -> b a", b=128))
            nc.sync.dma_start(ones[:], onesd[:])
            nc.sync.dma_start(mask[:], maskd[:])

            # ---- phase 1: projections ------------------------------------
            with (
                tc.tile_pool(name="xtp", bufs=1) as xtp,
                tc.tile_pool(name="wp", bufs=1) as wp,
                tc.tile_pool(name="psP", bufs=psp_bufs, space="PSUM") as psA,
            ):
                xts = [xtp.tile([128, T], F32R, name=f"xts{i}") for i in range(NDT)]
                wq_t = [wp.tile([128, FG], F32R, name=f"w{i}", tag=f"w{i}")
                        for i in range(NDT)]
                # interleave weight tiles with strip-0 x tiles so the
                # first psum accumulation can start after ~0.5MB of DMA
                for i in range(NDT):
                    nc.sync.dma_start(wq_t[i][:], wq[i * 128:(i + 1) * 128, :])
                    nc.sync.dma_start(
                        xts[i][:, 0:512], xt[i * 128:(i + 1) * 128, 0:512])
                for ts in range(1, NTS):
                    for i in range(NDT):
                        nc.sync.dma_start(
                            xts[i][:, ts * 512:(ts + 1) * 512],
                            xt[i * 128:(i + 1) * 128, ts * 512:(ts + 1) * 512])

                def emit_qk(wtl, dst):
                    for ts in range(NTS):
                        for ft in range(NFT):
                            ps = psA.tile([128, 512], F32, name="pj", tag="A")
                            for d in range(NDT):
                                nc.tensor.matmul(
                                    ps[:], wtl[d][:, ft * 128:(ft + 1) * 128],
                                    xts[d][:, ts * 512:(ts + 1) * 512],
                                    start=(d == 0), stop=(d == NDT - 1))
                            nc.vector.tensor_copy(
                                dst[ft][:, ts * 512:(ts + 1) * 512], ps[:])

                # Q^T first, then V (attention j=0 needs V'[0:4]), K^T last:
                # attention starts as soon as K^T strip 0 lands.
                emit_qk(wq_t, qt)

                wvl = [wp.tile([128, FG], F32R, name=f"wv{i}", tag=f"w{i}")
                       for i in range(NDT)]
                for i in range(NDT):
                    nc.sync.dma_start(wvl[i][:], wv[i * 128:(i + 1) * 128, :])
                for tt in range(NTT):
                    ps = psA.tile([128, 512], F32, name="pv", tag="A")
                    for d in range(NDT):
                        nc.tensor.matmul(
                            ps[:], xts[d][:, tt * 128:(tt + 1) * 128], wvl[d][:],
                            start=(d == 0), stop=(d == NDT - 1))
                    vv = vp[tt].rearrange("p (h c) -> p h c", h=8)
                    pv = ps.rearrange("p (h c) -> p h c", h=8)
                    nc.vector.tensor_copy(vv[:, :, 0:64], pv[:])
                    nc.sync.dma_start(
                        vv[:, :, 64:65],
                        onescol.rearrange("p (h c) -> p h c", h=16)[:, 0:8, :])

                wkl = [wp.tile([128, FG], F32R, name=f"wk{i}", tag=f"w{i}")
                       for i in range(NDT)]
                for i in range(NDT):
                    nc.sync.dma_start(wkl[i][:], wk[i * 128:(i + 1) * 128, :])
                emit_qk(wkl, kt)

            # ---- phase 2: attention + output projection ------------------
            with (
                tc.tile_pool(name="otp", bufs=1) as otp,
                tc.tile_pool(name="wop", bufs=1) as wop,
                tc.tile_pool(name="psA", bufs=psa_bufs, space="PSUM") as psA,
                tc.tile_pool(name="psB", bufs=psb_bufs, space="PSUM") as psB,
                tc.tile_pool(name="psC", bufs=psc_bufs, space="PSUM") as psC,
            ):
                ot = [otp.tile([128, T], F32R, name=f"ot{i}") for i in range(NFT)]
                wot = [wop.tile([128, D_MODEL], F32R, name=f"wot{i}")
                       for i in range(NFT)]
                for i in range(NFT):
                    nc.sync.dma_start(wot[i][:], wo[i * 128:(i + 1) * 128, :])

                for j in (j_order or range(NTS)):
                    for h in range(8):
                        ft, base = h // 2, (h % 2) * 64
                        if causal:
                            nkt = 4 * j + 4    # causal key tiles for strip j
                            if KORDER_MODE == "diag_first":
                                korder = list(range(4 * j, nkt)) + list(range(0, 4 * j))
                            elif KORDER_MODE == "diag_last":
                                korder = [4 * j] + list(range(0, 4 * j)) + list(range(4 * j + 1, nkt))
                            else:  # ascending
                                korder = list(range(nkt))
                        else:
                            nkt = NTT
                            korder = list(range(NTT))
                        ops = psB.tile([128, 512], F32, name="ops", tag="B")

                        def _lo(kt_):
                            if not causal:
                                return 0
                            m_ = kt_ - 4 * j
                            return 128 * m_ if m_ > 0 else 0

                        nb = act_width // 512
                        ki = 0
                        while ki < nkt:
                            gts = korder[ki:ki + nb]
                            if any(_lo(k_) for k_ in gts):
                                gts = gts[:1]
                            gn = len(gts)
                            lo0 = _lo(gts[0])
                            sps = psA.tile([128, act_width], F32, name="sps", tag="A")
                            se = stp.tile([128, act_width], F32R, name="se", tag="se")
                            for u, ktile in enumerate(gts):
                                lo = _lo(ktile)
                                nc.tensor.matmul(
                                    sps[:, u * 512 + lo:(u + 1) * 512],
                                    kt[ft][base:base + 64, ktile * 128:(ktile + 1) * 128],
                                    qt[ft][base:base + 64, j * 512 + lo:(j + 1) * 512],
                                    start=True, stop=True)
                            if gn == nb and nb > 1:
                                nc.scalar.activation(
                                    se[:, 0:gn * 512], sps[:, 0:gn * 512], AF.Exp,
                                    bias=cb[:, gts[0]:gts[0] + 1], scale=0.125)
                            else:
                                for u, ktile in enumerate(gts):
                                    lo = _lo(ktile)
                                    nc.scalar.activation(
                                        se[:, u * 512 + lo:(u + 1) * 512],
                                        sps[:, u * 512 + lo:(u + 1) * 512], AF.Exp,
                                        bias=cb[:, ktile:ktile + 1], scale=0.125)
                            for u, ktile in enumerate(gts):
                                m = (ktile - 4 * j) if causal else -1
                                if m >= 0:
                                    lo = _lo(ktile)
                                    nc.vector.tensor_mul(
                                        se[:, u * 512 + lo:(u + 1) * 512],
                                        se[:, u * 512 + lo:(u + 1) * 512],
                                        mask[:, 0: 512 - 128 * m])
                            for u, ktile in enumerate(gts):
                                lo = _lo(ktile)
                                nc.tensor.matmul(
                                    ops[0:65, lo:512],
                                    vp[ktile][:, h * 65:(h + 1) * 65],
                                    se[:, u * 512 + lo:(u + 1) * 512],
                                    start=(ki + u == 0), stop=(ki + u == nkt - 1))
                            ki += gn
                        # normalize rows 0:64 by row 64 and store to OT
                        r = msc.tile([1, 512], F32R, name="r", tag="r")
                        nc.vector.reciprocal(r[:], ops[64:65, :])
                        rps = psC.tile([64, 512], F32, name="rps", tag="C")
                        nc.tensor.matmul(rps[:], ones[:], r[:],
                                         start=True, stop=True)
                        rb = msc.tile([64, 512], F32, name="rb", tag="rb")
                        nc.vector.tensor_copy(rb[:], rps[:])
                        nc.vector.tensor_mul(
                            ot[ft][base:base + 64, j * 512:(j + 1) * 512],
                            ops[0:64, :], rb[:])

                    # y = OT^T @ woT; optionally deferred one strip to
                    # dodge the diagonal-heavy DVE window
                    yjs = ([j - 1] if j > 0 else []) if y_defer else [j]
                    if y_defer and j == NTS - 1:
                        yjs = [j - 1, j]
                    for yj in yjs:
                      for tt in range(4 * yj, 4 * yj + 4):
                        for nn in range(2):
                            yps = psC.tile([128, 512], F32, name="yps", tag="C")
                            for ft in range(NFT):
                                nc.tensor.matmul(
                                    yps[:], ot[ft][:, tt * 128:(tt + 1) * 128],
                                    wot[ft][:, nn * 512:(nn + 1) * 512],
                                    start=(ft == 0), stop=(ft == NFT - 1))
                            ysb = msc.tile([128, 512], F32, name="ysb", tag="y")
                            nc.vector.tensor_copy(ysb[:], yps[:])
                            nc.sync.dma_start(
                                y[tt * 128:(tt + 1) * 128, nn * 512:(nn + 1) * 512],
                                ysb[:])

    _orig = nc.to_json_bytes
    nc.to_json_bytes = lambda: split_multiwaits(_orig())
    return nc


_NC = {}


def _get_nc(causal=True) -> bass.Bass:
    if causal not in _NC:
        _NC[causal] = build_nc(causal=causal)
    return _NC[causal]


# ---------------------------------------------------------------------------
# Host-side sharding + gather
# ---------------------------------------------------------------------------

def _kernel_numpy(q, mask, Wq, bq, Wk, bk, Wv, bv, Wo, bo):
    """Exact host fallback for unexpected shapes or arbitrary masks."""
    b, t, d = q.shape
    h = N_HEADS if d == D_MODEL else max(1, d // D_K)
    dk = d // h
    qh = (q @ Wq.T + bq).reshape(b, t, h, dk).transpose(0, 2, 1, 3)
    kh = (q @ Wk.T + bk).reshape(b, t, h, dk).transpose(0, 2, 1, 3)
    vh = (q @ Wv.T + bv).reshape(b, t, h, dk).transpose(0, 2, 1, 3)
    s = np.einsum("bhqd,bhkd->bhqk", qh, kh) / np.sqrt(dk).astype(np.float32)
    s = np.where(mask, -np.inf, s)
    s = s - s.max(axis=-1, keepdims=True)
    p = np.exp(s)
    p /= p.sum(axis=-1, keepdims=True)
    o = np.einsum("bhqk,bhkd->bhqd", p, vh)
    o = o.transpose(0, 2, 1, 3).reshape(b, t, d)
    return (o @ Wo.T + bo).astype(np.float32)


_CAUSAL_REF = None


def _mask_kind(mask):
    """Classify the (B,1,T,T) bool mask: "causal" / "none" / "other"."""
    global _CAUSAL_REF
    mask = np.asarray(mask)
    if mask.shape != (B, 1, T, T):
        return "other"
    if not mask.any():
        return "none"
    if _CAUSAL_REF is None:
        _CAUSAL_REF = np.triu(np.ones((T, T), dtype=bool), k=1)
    for i in range(mask.shape[0]):
        if not np.array_equal(mask[i, 0], _CAUSAL_REF):
            return "other"
    return "causal"


def kernel(q, mask, Wq, bq, Wk, bk, Wv, bv, Wo, bo):
    q = np.asarray(q, np.float32)
    Wq = np.asarray(Wq, np.float32); bq = np.asarray(bq, np.float32)
    Wk = np.asarray(Wk, np.float32); bk = np.asarray(bk, np.float32)
    Wv = np.asarray(Wv, np.float32); bv = np.asarray(bv, np.float32)
    Wo = np.asarray(Wo, np.float32); bo = np.asarray(bo, np.float32)

    kind = _mask_kind(mask)
    if q.shape != (B, T, D_MODEL) or Wq.shape != (D_MODEL, D_MODEL) or kind == "other":
        return _kernel_numpy(np.asarray(q, np.float32), np.asarray(mask, bool),
                             Wq, bq, Wk, bk, Wv, bv, Wo, bo)

    onesd = np.ones((1, 64), np.float32)
    onescol = np.ones((128, 16), np.float32)
    # causal staircase: M[k, u] = 1 iff k <= u (used as [0 : 512-128m])
    kk = np.arange(128)[:, None]
    vv_ = np.arange(512)[None, :]
    maskst = (kk <= vv_).astype(np.float32)

    in_maps = []
    for b in range(B):
        x = np.ascontiguousarray(q[b])                    # (T, D)
        xT = np.ascontiguousarray(x.T)                    # (D, T)
        # per-key exp bias: bq . K_k term (softmax-relevant) minus shift M0
        cvec = ((x @ (Wk.T @ bq) + float(bq @ bk)) * 0.125 - M0).astype(np.float32)
        for g in range(2):
            sl = slice(g * FG, (g + 1) * FG)
            in_maps.append({
                "xt": xT,
                "wq": np.ascontiguousarray(Wq[sl].T),     # (D, FG)
                "wk": np.ascontiguousarray(Wk[sl].T),
                "wv": np.ascontiguousarray(Wv[sl].T),
                "wo": np.ascontiguousarray(Wo[:, sl].T),  # (FG, D)
                "cvec": cvec,
                "onesd": onesd,
                "onescol": onescol,
                "maskd": maskst,
            })

    nc = _get_nc(causal=(kind == "causal"))
    res = bass_utils.run_bass_kernel_spmd(nc, in_maps, core_ids=list(range(N_CORES)))

    out = np.empty((B, T, D_MODEL), np.float32)
    # exact bias correction terms (zero when biases are zero):
    # V-bias contributes bv @ Wo.T (softmax rows sum to 1); plus bo.
    corr = (bv @ Wo.T + bo).astype(np.float32)
    for b in range(B):
        out[b] = res.results[2 * b]["y"] + res.results[2 * b + 1]["y"] + corr
    return out


# revision 28
# speedup vs baseline: 1.0182x; 1.0173x over previous
"""Multi-head attention (B=4, T=2048, D=1024, H=16 heads, causal) on 8
Trainium2 NeuronCores.

Sharding: core i handles batch b = i//2 and head-group g = i%2 (8 heads,
512 features). Each core computes its head-group's attention output and a
partial output projection; the host sums the two partials per batch.

All heavy matmuls run as float32r (fp32 storage, ~12-bit-mantissa multiply)
which streams at full PE rate for free-dim >= 256.

Attention is computed entirely in "transposed score" layout to avoid any
on-device transposes:
  ST[k, q] = K_k . Q_q   (keys on partitions, queries on free dim)
  P^T = exp(ST/8 - M0) * causal_mask   (ACT exp, per-partition bias slot)
  [OT; s] = V'^T @ P^T   where V' has a ones column, so the softmax
            denominator s comes out of the same matmul (row 64).
  OT_norm = OT * (1/s)   (DVE reciprocal + PE rank-1 broadcast of 1/s)
  y_partial = OT_norm^T @ Wo^T-slice  (natural layout, DMA'd out directly)
"""

import numpy as np

import concourse.bass as bass
import concourse.mybir as mybir
import concourse.tile as tile
from concourse import bass_utils

import orjson

D_MODEL = 1024
N_HEADS = 16
D_K = 64
B, T = 4, 2048
FG = 512            # features per head-group (8 heads x 64)
N_CORES = 8
M0 = 12.0           # global exp shift (cancels exactly in softmax)
KORDER_MODE = "diag_last"

F32R = mybir.dt.float32r
F32 = mybir.dt.float32
AF = mybir.ActivationFunctionType


# ---------------------------------------------------------------------------
# BIR legalization for the stock walrus in this container: it encodes at most
# ONE sync wait per instruction, so spill extra waits onto single-wait
# EventSemaphore nops placed immediately before the instruction on the same
# engine.  Also statically verify no wait precedes (in program order) the
# instructions that produce its semaphore value, which would deadlock.
# ---------------------------------------------------------------------------

def split_multiwaits(bir_bytes: bytes) -> bytes:
    bir = orjson.loads(bir_bytes)
    n = [0]
    for fn in bir.get("functions", []):
        for blk in fn.get("blocks", []):
            out = []
            for inst in blk.get("instructions", []):
                si = inst.get("sync_info") or {}
                waits = si.get("on_wait") or []
                if len(waits) > 1:
                    for w in waits[:-1]:
                        n[0] += 1
                        out.append({
                            "debug": inst.get("debug", 0),
                            "engine": inst["engine"],
                            "ins": [], "name": f"WSPLIT-{n[0]}",
                            "opcode": "EventSemaphore", "outs": [],
                            "sync_info": {"on_update": [], "on_wait": [w]},
                        })
                    si["on_wait"] = waits[-1:]
                    inst["sync_info"] = si
                out.append(inst)
            blk["instructions"] = out
    _check_no_forward_waits(bir)
    return orjson.dumps(bir)


def _check_no_forward_waits(bir) -> None:
    issued = {}
    for fn in bir.get("functions", []):
        for blk in fn.get("blocks", []):
            for inst in blk.get("instructions", []):
                si = inst.get("sync_info") or {}
                for w in (si.get("on_wait") or []):
                    if (w.get("sync_type") == "semaphore"
                            and w.get("wait_mode") == "sem-ge-imm"
                            and "barrier" not in (w.get("ant_name") or "")):
                        if issued.get(w["id"], 0) < w["wait_value"]:
                            raise RuntimeError(
                                f"forward wait: {inst['name']} waits "
                                f"{w.get('ant_name')}>={w['wait_value']}")
                for u in (si.get("on_update") or []):
                    if (u.get("sync_type") == "semaphore"
                            and u.get("update_mode") in ("sem-inc", "sem-add-imm")):
                        issued[u["id"]] = issued.get(u["id"], 0) + u.get("update_value", 1)


# ---------------------------------------------------------------------------
# Device program (identical on all 8 cores; data differs per core)
# ---------------------------------------------------------------------------

def build_nc(st_bufs=8, psa_bufs=4, exp_batch=1, batch_act=False, pool_mask=False, psb_bufs=2, psc_bufs=1, msc_bufs=2, act_width=512, y_from_psum=False, y_defer=False, psp_bufs=8, j_order=None, causal=True) -> bass.Bass:
    nc = bass.Bass("TRN2", target_bir_lowering=False, debug=False)

    xt = nc.dram_tensor("xt", (D_MODEL, T), F32R, kind="ExternalInput")
    wq = nc.dram_tensor("wq", (D_MODEL, FG), F32R, kind="ExternalInput")
    wk = nc.dram_tensor("wk", (D_MODEL, FG), F32R, kind="ExternalInput")
    wv = nc.dram_tensor("wv", (D_MODEL, FG), F32R, kind="ExternalInput")
    wo = nc.dram_tensor("wo", (FG, D_MODEL), F32R, kind="ExternalInput")
    cvec = nc.dram_tensor("cvec", (T,), F32, kind="ExternalInput")
    onesd = nc.dram_tensor("onesd", (1, 64), F32R, kind="ExternalInput")
    onescol = nc.dram_tensor("onescol", (128, 16), F32R, kind="ExternalInput")
    maskd = nc.dram_tensor("maskd", (128, 512), F32R, kind="ExternalInput")
    y = nc.dram_tensor("y", (T, D_MODEL), F32, kind="ExternalOutput")

    NDT = D_MODEL // 128   # 8 contraction tiles
    NFT = FG // 128        # 4 feature tiles (2 heads each)
    NTT = T // 128         # 16 token tiles
    NTS = T // 512         # 4 token strips

    with tile.TileContext(nc) as tc, nc.allow_low_precision(reason="f32r storage"):
        with (
            tc.tile_pool(name="qtp", bufs=1) as qtp,
            tc.tile_pool(name="ktp", bufs=1) as ktp,
            tc.tile_pool(name="vpp", bufs=1) as vpp,
            tc.tile_pool(name="stp", bufs=st_bufs) as stp,
            tc.tile_pool(name="cst", bufs=1) as cst,
            tc.tile_pool(name="msc", bufs=msc_bufs) as msc,
        ):
            # persistent tiles
            qt = [qtp.tile([128, T], F32R, name=f"qt{i}") for i in range(NFT)]
            kt = [ktp.tile([128, T], F32R, name=f"kt{i}") for i in range(NFT)]
            vp = [vpp.tile([128, 520], F32R, name=f"vp{i}") for i in range(NTT)]
            cb = cst.tile([128, 16], F32, name="cb")
            ones = cst.tile([1, 64], F32R, name="ones")
            mask = cst.tile([128, 512], F32R, name="mask")
            nc.sync.dma_start(cb[:], cvec.rearrange("(a b) -> b a", b=128))
            nc.sync.dma_start(ones[:], onesd[:])
            nc.sync.dma_start(mask[:], maskd[:])

            # ---- phase 1: projections ------------------------------------
            with (
                tc.tile_pool(name="xtp", bufs=1) as xtp,
                tc.tile_pool(name="wp", bufs=1) as wp,
                tc.tile_pool(name="psP", bufs=psp_bufs, space="PSUM") as psA,
            ):
                xts = [xtp.tile([128, T], F32R, name=f"xts{i}") for i in range(NDT)]
                wq_t = [wp.tile([128, FG], F32R, name=f"w{i}", tag=f"w{i}")
                        for i in range(NDT)]
                # interleave weight tiles with strip-0 x tiles so the
                # first psum accumulation can start after ~0.5MB of DMA
                for i in range(NDT):
                    nc.sync.dma_start(wq_t[i][:], wq[i * 128:(i + 1) * 128, :])
                    nc.sync.dma_start(
                        xts[i][:, 0:512], xt[i * 128:(i + 1) * 128, 0:512])
                for ts in range(1, NTS):
                    for i in range(NDT):
                        nc.sync.dma_start(
                            xts[i][:, ts * 512:(ts + 1) * 512],
                            xt[i * 128:(i + 1) * 128, ts * 512:(ts + 1) * 512])

                def emit_qk(wtl, dst):
                    for ts in range(NTS):
                        for ft in range(NFT):
                            ps = psA.tile([128, 512], F32, name="pj", tag="A")
                            for d in range(NDT):
                                nc.tensor.matmul(
                                    ps[:], wtl[d][:, ft * 128:(ft + 1) * 128],
                                    xts[d][:, ts * 512:(ts + 1) * 512],
                                    start=(d == 0), stop=(d == NDT - 1))
                            nc.vector.tensor_copy(
                                dst[ft][:, ts * 512:(ts + 1) * 512], ps[:])

                # Q^T first, then V (attention j=0 needs V'[0:4]), K^T last:
                # attention starts as soon as K^T strip 0 lands.
                emit_qk(wq_t, qt)

                wvl = [wp.tile([128, FG], F32R, name=f"wv{i}", tag=f"w{i}")
                       for i in range(NDT)]
                for i in range(NDT):
                    nc.sync.dma_start(wvl[i][:], wv[i * 128:(i + 1) * 128, :])
                for tt in range(NTT):
                    ps = psA.tile([128, 512], F32, name="pv", tag="A")
                    for d in range(NDT):
                        nc.tensor.matmul(
                            ps[:], xts[d][:, tt * 128:(tt + 1) * 128], wvl[d][:],
                            start=(d == 0), stop=(d == NDT - 1))
                    vv = vp[tt].rearrange("p (h c) -> p h c", h=8)
                    pv = ps.rearrange("p (h c) -> p h c", h=8)
                    nc.vector.tensor_copy(vv[:, :, 0:64], pv[:])
                    nc.sync.dma_start(
                        vv[:, :, 64:65],
                        onescol.rearrange("p (h c) -> p h c", h=16)[:, 0:8, :])

                wkl = [wp.tile([128, FG], F32R, name=f"wk{i}", tag=f"w{i}")
                       for i in range(NDT)]
                for i in range(NDT):
                    nc.sync.dma_start(wkl[i][:], wk[i * 128:(i + 1) * 128, :])
                emit_qk(wkl, kt)

            # ---- phase 2: attention + output projection ------------------
            with (
                tc.tile_pool(name="otp", bufs=1) as otp,
                tc.tile_pool(name="wop", bufs=1) as wop,
                tc.tile_pool(name="psA", bufs=psa_bufs, space="PSUM") as psA,
                tc.tile_pool(name="psB", bufs=psb_bufs, space="PSUM") as psB,
                tc.tile_pool(name="psC", bufs=psc_bufs, space="PSUM") as psC,
            ):
                ot = [otp.tile([128, T], F32R, name=f"ot{i}") for i in range(NFT)]
                wot = [wop.tile([128, D_MODEL], F32R, name=f"wot{i}")
                       for i in range(NFT)]
                for i in range(NFT):
                    nc.sync.dma_start(wot[i][:], wo[i * 128:(i + 1) * 128, :])

                for j in (j_order or range(NTS)):
                    for h in range(8):
                        ft, base = h // 2, (h % 2) * 64
                        if causal:
                            nkt = 4 * j + 4    # causal key tiles for strip j
                            if KORDER_MODE == "diag_first":
                                korder = list(range(4 * j, nkt)) + list(range(0, 4 * j))
                            elif KORDER_MODE == "diag_last":
                                korder = [4 * j] + list(range(0, 4 * j)) + list(range(4 * j + 1, nkt))
                            else:  # ascending
                                korder = list(range(nkt))
                        else:
                            nkt = NTT
                            korder = list(range(NTT))
                        ops = psB.tile([128, 512], F32, name="ops", tag="B")

                        def _lo(kt_):
                            if not causal:
                                return 0
                            m_ = kt_ - 4 * j
                            return 128 * m_ if m_ > 0 else 0

                        nb = act_width // 512
                        ki = 0
                        while ki < nkt:
                            gts = korder[ki:ki + nb]
                            if any(_lo(k_) for k_ in gts):
                                gts = gts[:1]
                            gn = len(gts)
                            lo0 = _lo(gts[0])
                            sps = psA.tile([128, act_width], F32, name="sps", tag="A")
                            se = stp.tile([128, act_width], F32R, name="se", tag="se")
                            for u, ktile in enumerate(gts):
                                lo = _lo(ktile)
                                nc.tensor.matmul(
                                    sps[:, u * 512 + lo:(u + 1) * 512],
                                    kt[ft][base:base + 64, ktile * 128:(ktile + 1) * 128],
                                    qt[ft][base:base + 64, j * 512 + lo:(j + 1) * 512],
                                    start=True, stop=True)
                            if gn == nb and nb > 1:
                                nc.scalar.activation(
                                    se[:, 0:gn * 512], sps[:, 0:gn * 512], AF.Exp,
                                    bias=cb[:, gts[0]:gts[0] + 1], scale=0.125)
                            else:
                                for u, ktile in enumerate(gts):
                                    lo = _lo(ktile)
                                    nc.scalar.activation(
                                        se[:, u * 512 + lo:(u + 1) * 512],
                                        sps[:, u * 512 + lo:(u + 1) * 512], AF.Exp,
                                        bias=cb[:, ktile:ktile + 1], scale=0.125)
                            for u, ktile in enumerate(gts):
                                m = (ktile - 4 * j) if causal else -1
                                if m >= 0:
                                    lo = _lo(ktile)
                                    nc.vector.tensor_mul(
                                        se[:, u * 512 + lo:(u + 1) * 512],
                                        se[:, u * 512 + lo:(u + 1) * 512],
                                        mask[:, 0: 512 - 128 * m])
                            for u, ktile in enumerate(gts):
                                lo = _lo(ktile)
                                nc.tensor.matmul(
                                    ops[0:65, lo:512],
                                    vp[ktile][:, h * 65:(h + 1) * 65],
                                    se[:, u * 512 + lo:(u + 1) * 512],
                                    start=(ki + u == 0), stop=(ki + u == nkt - 1))
                            ki += gn
                        # normalize rows 0:64 by row 64 and store to OT
                        r = msc.tile([1, 512], F32R, name="r", tag="r")
                        nc.vector.reciprocal(r[:], ops[64:65, :])
                        rps = psC.tile([64, 512], F32, name="rps", tag="R")
                        nc.tensor.matmul(rps[:], ones[:], r[:],
                                         start=True, stop=True)
                        rb = msc.tile([64, 512], F32, name="rb", tag="rb")
                        nc.vector.tensor_copy(rb[:], rps[:])
                        nc.vector.tensor_mul(
                            ot[ft][base:base + 64, j * 512:(j + 1) * 512],
                            ops[0:64, :], rb[:])

                    # y = OT^T @ woT; optionally deferred one strip to
                    # dodge the diagonal-heavy DVE window
                    yjs = ([j - 1] if j > 0 else []) if y_defer else [j]
                    if y_defer and j == NTS - 1:
                        yjs = [j - 1, j]
                    for yj in yjs:
                      for tt in range(4 * yj, 4 * yj + 4):
                        for nn in range(2):
                            yps = psC.tile([128, 512], F32, name="yps", tag="C")
                            for ft in range(NFT):
                                nc.tensor.matmul(
                                    yps[:], ot[ft][:, tt * 128:(tt + 1) * 128],
                                    wot[ft][:, nn * 512:(nn + 1) * 512],
                                    start=(ft == 0), stop=(ft == NFT - 1))
                            ysb = msc.tile([128, 512], F32, name="ysb", tag="y")
                            nc.vector.tensor_copy(ysb[:], yps[:])
                            nc.sync.dma_start(
                                y[tt * 128:(tt + 1) * 128, nn * 512:(nn + 1) * 512],
                                ysb[:])

    _orig = nc.to_json_bytes
    nc.to_json_bytes = lambda: split_multiwaits(_orig())
    return nc


_NC = {}


def _get_nc(causal=True) -> bass.Bass:
    if causal not in _NC:
        _NC[causal] = build_nc(causal=causal)
    return _NC[causal]


# ---------------------------------------------------------------------------
# Host-side sharding + gather
# ---------------------------------------------------------------------------

def _kernel_numpy(q, mask, Wq, bq, Wk, bk, Wv, bv, Wo, bo):
    """Exact host fallback for unexpected shapes or arbitrary masks."""
    b, t, d = q.shape
    h = N_HEADS if d == D_MODEL else max(1, d // D_K)
    dk = d // h
    qh = (q @ Wq.T + bq).reshape(b, t, h, dk).transpose(0, 2, 1, 3)
    kh = (q @ Wk.T + bk).reshape(b, t, h, dk).transpose(0, 2, 1, 3)
    vh = (q @ Wv.T + bv).reshape(b, t, h, dk).transpose(0, 2, 1, 3)
    s = np.einsum("bhqd,bhkd->bhqk", qh, kh) / np.sqrt(dk).astype(np.float32)
    s = np.where(mask, -np.inf, s)
    s = s - s.max(axis=-1, keepdims=True)
    p = np.exp(s)
    p /= p.sum(axis=-1, keepdims=True)
    o = np.einsum("bhqk,bhkd->bhqd", p, vh)
    o = o.transpose(0, 2, 1, 3).reshape(b, t, d)
    return (o @ Wo.T + bo).astype(np.float32)


_CAUSAL_REF = None


def _mask_kind(mask):
    """Classify the (B,1,T,T) bool mask: "causal" / "none" / "other"."""
    global _CAUSAL_REF
    mask = np.asarray(mask)
    if mask.shape != (B, 1, T, T):
        return "other"
    if not mask.any():
        return "none"
    if _CAUSAL_REF is None:
        _CAUSAL_REF = np.triu(np.ones((T, T), dtype=bool), k=1)
    for i in range(mask.shape[0]):
        if not np.array_equal(mask[i, 0], _CAUSAL_REF):
            return "other"
    return "causal"


def kernel(q, mask, Wq, bq, Wk, bk, Wv, bv, Wo, bo):
    q = np.asarray(q, np.float32)
    Wq = np.asarray(Wq, np.float32); bq = np.asarray(bq, np.float32)
    Wk = np.asarray(Wk, np.float32); bk = np.asarray(bk, np.float32)
    Wv = np.asarray(Wv, np.float32); bv = np.asarray(bv, np.float32)
    Wo = np.asarray(Wo, np.float32); bo = np.asarray(bo, np.float32)

    kind = _mask_kind(mask)
    if q.shape != (B, T, D_MODEL) or Wq.shape != (D_MODEL, D_MODEL) or kind == "other":
        return _kernel_numpy(np.asarray(q, np.float32), np.asarray(mask, bool),
                             Wq, bq, Wk, bk, Wv, bv, Wo, bo)

    onesd = np.ones((1, 64), np.float32)
    onescol = np.ones((128, 16), np.float32)
    # causal staircase: M[k, u] = 1 iff k <= u (used as [0 : 512-128m])
    kk = np.arange(128)[:, None]
    vv_ = np.arange(512)[None, :]
    maskst = (kk <= vv_).astype(np.float32)

    in_maps = []
    for b in range(B):
        x = np.ascontiguousarray(q[b])                    # (T, D)
        xT = np.ascontiguousarray(x.T)                    # (D, T)
        # per-key exp bias: bq . K_k term (softmax-relevant) minus shift M0
        cvec = ((x @ (Wk.T @ bq) + float(bq @ bk)) * 0.125 - M0).astype(np.float32)
        for g in range(2):
            sl = slice(g * FG, (g + 1) * FG)
            in_maps.append({
                "xt": xT,
                "wq": np.ascontiguousarray(Wq[sl].T),     # (D, FG)
                "wk": np.ascontiguousarray(Wk[sl].T),
                "wv": np.ascontiguousarray(Wv[sl].T),
                "wo": np.ascontiguousarray(Wo[:, sl].T),  # (FG, D)
                "cvec": cvec,
                "onesd": onesd,
                "onescol": onescol,
                "maskd": maskst,
            })

    nc = _get_nc(causal=(kind == "causal"))
    res = bass_utils.run_bass_kernel_spmd(nc, in_maps, core_ids=list(range(N_CORES)))

    out = np.empty((B, T, D_MODEL), np.float32)
    # exact bias correction terms (zero when biases are zero):
    # V-bias contributes bv @ Wo.T (softmax rows sum to 1); plus bo.
    corr = (bv @ Wo.T + bo).astype(np.float32)
    for b in range(B):
        out[b] = res.results[2 * b]["y"] + res.results[2 * b + 1]["y"] + corr
    return out
